# revision 1
# baseline (speedup 1.0000x reference)
"""CBOW word2vec forward-loss kernel for 8 Trainium2 NeuronCores.

Strategy (data-parallel, per sharding hint):
  - Batch B=131072 is split across 8 cores (16384 samples each); the two
    embedding tables are replicated to every core.
  - On each core, samples are laid out as [128 partitions x 128 tiles].
    Per super-tile of 4 sample-tiles, two large indirect-DMA gathers pull
    the context rows (emb0) and the word+negative rows (emb1) into SBUF
    (one 512B descriptor per embedding row, batched thousands of
    descriptors per instruction to amortize SWDGE overhead).
  - DVE sums the 10 context rows (binary tree), multiplies with the
    word/neg rows (broadcast AP) and reduces over D=128 to per-sample
    inner products.
  - Tail: ips * (+-1/ctx_len), clip to +-10, softplus via Exp + Ln(1+x)
    on the scalar engine with a fused per-partition accumulate.
  - Each core writes back [128] partial sums; the host adds the 1024
    partials into the final scalar.
"""

import numpy as np

import concourse.bacc as bacc
import concourse.bass as bass
import concourse.mybir as mybir
import concourse.tile as tile
from concourse.bass_utils import run_bass_kernel_spmd

P = 128          # partitions / samples per tile
D = 128          # embedding dim
C = 10           # context slots
NNEG = 5         # negatives
SLOTS = 1 + NNEG # word + negatives gathered from emb1
V0 = 100001      # emb0 rows (incl. padding row)
V1 = 100000      # emb1 rows
B = 131072       # full batch
M = 8            # cores
BC = B // M      # samples per core
T = BC // P      # sample-tiles per core (128)
S = 4            # sample-tiles per super-tile (gather batch)
G = T // S       # super-tiles per core

F32 = mybir.dt.float32
I32 = mybir.dt.int32


def build_nc(t_tiles=T, s_tiles=S):
    """Emit the single-core Bass program (run SPMD on all 8 cores)."""
    t, s = t_tiles, s_tiles
    g_iters = t // s
    nc = bacc.Bacc("TRN2", target_bir_lowering=False, debug=False)

    emb0 = nc.dram_tensor("emb0", [V0, D], F32, kind="ExternalInput")
    emb1 = nc.dram_tensor("emb1", [V1, D], F32, kind="ExternalInput")
    ctx_idx = nc.dram_tensor("ctx_idx", [P, t * C], I32, kind="ExternalInput")
    wn_idx = nc.dram_tensor("wn_idx", [P, t * SLOTS], I32, kind="ExternalInput")
    lens = nc.dram_tensor("lens", [P, t], F32, kind="ExternalInput")
    out = nc.dram_tensor("out", [P, 1], F32, kind="ExternalOutput")

    with tile.TileContext(nc) as tc:
        with (
            tc.tile_pool(name="persist", bufs=1) as pp,
            tc.tile_pool(name="gather", bufs=2) as gp,
            tc.tile_pool(name="work", bufs=2) as wp,
        ):
            ctx_idx_sb = pp.tile([P, t * C], I32)
            wn_idx_sb = pp.tile([P, t * SLOTS], I32)
            lens_sb = pp.tile([P, t], F32)
            nc.sync.dma_start(ctx_idx_sb[:, :], ctx_idx.ap()[:, :])
            nc.sync.dma_start(wn_idx_sb[:, :], wn_idx.ap()[:, :])
            nc.sync.dma_start(lens_sb[:, :], lens.ap()[:, :])

            # scl[p, t, j] = -1/len for j==0 (word), +1/len for j>0 (negs)
            rlen = pp.tile([P, t], F32)
            nc.vector.reciprocal(rlen[:, :], lens_sb[:, :])
            scl = pp.tile([P, t * SLOTS], F32)
            scl_v = scl[:, :].rearrange("p (t s) -> p t s", s=SLOTS)
            rlen_v = rlen[:, :].rearrange("p (t s) -> p t s", s=1)
            nc.vector.tensor_scalar_mul(scl_v[:, :, 0:1], rlen_v, -1.0)
            nc.vector.tensor_copy(
                scl_v[:, :, 1:SLOTS], rlen_v.broadcast_to((P, t, NNEG))
            )

            ips = pp.tile([P, t * SLOTS], F32)

            for gi in range(g_iters):
                cg = gp.tile([P, s * C * D], F32, tag="cg")
                wng = gp.tile([P, s * SLOTS * D], F32, tag="wng")
                nc.gpsimd.indirect_dma_start(
                    out=cg[:, :],
                    out_offset=None,
                    in_=emb0.ap()[:, :],
                    in_offset=bass.IndirectOffsetOnAxis(
                        ap=ctx_idx_sb[:, gi * s * C : (gi + 1) * s * C], axis=0
                    ),
                )
                nc.gpsimd.indirect_dma_start(
                    out=wng[:, :],
                    out_offset=None,
                    in_=emb1.ap()[:, :],
                    in_offset=bass.IndirectOffsetOnAxis(
                        ap=wn_idx_sb[:, gi * s * SLOTS : (gi + 1) * s * SLOTS],
                        axis=0,
                    ),
                )

                cg4 = cg[:, :].rearrange("p (s c d) -> p s c d", s=s, c=C, d=D)
                a = wp.tile([P, s * 5 * D], F32, tag="a")
                a4 = a[:, :].rearrange("p (s c d) -> p s c d", s=s, c=5, d=D)
                nc.vector.tensor_add(a4, cg4[:, :, 0:5, :], cg4[:, :, 5:10, :])
                b = wp.tile([P, s * 2 * D], F32, tag="b")
                b4 = b[:, :].rearrange("p (s c d) -> p s c d", s=s, c=2, d=D)
                nc.vector.tensor_add(b4, a4[:, :, 0:2, :], a4[:, :, 2:4, :])
                c1 = wp.tile([P, s * D], F32, tag="c1")
                c14 = c1[:, :].rearrange("p (s c d) -> p s c d", s=s, c=1, d=D)
                nc.vector.tensor_add(c14, b4[:, :, 0:1, :], a4[:, :, 4:5, :])
                csum = wp.tile([P, s * D], F32, tag="csum")
                csum4 = csum[:, :].rearrange("p (s c d) -> p s c d", s=s, c=1, d=D)
                nc.vector.tensor_add(csum4, c14, b4[:, :, 1:2, :])

                prod = wp.tile([P, s * SLOTS * D], F32, tag="prod")
                prod4 = prod[:, :].rearrange(
                    "p (s k d) -> p s k d", s=s, k=SLOTS, d=D
                )
                wng4 = wng[:, :].rearrange("p (s k d) -> p s k d", s=s, k=SLOTS, d=D)
                csum_b = (
                    csum[:, :]
                    .rearrange("p (s k d) -> p s k d", s=s, k=1, d=D)
                    .broadcast_to((P, s, SLOTS, D))
                )
                nc.vector.tensor_mul(prod4, wng4, csum_b)
                nc.vector.tensor_reduce(
                    ips[:, gi * s * SLOTS : (gi + 1) * s * SLOTS],
                    prod[:, :].rearrange("p (g d) -> p g d", d=D),
                    axis=mybir.AxisListType.X,
                    op=mybir.AluOpType.add,
                )

            # tail: scale by +-1/len, clip, softplus, fused partition-sum
            sc = pp.tile([P, t * SLOTS], F32)
            nc.vector.tensor_mul(sc[:, :], ips[:, :], scl[:, :])
            nc.vector.tensor_scalar_min(sc[:, :], sc[:, :], 10.0)
            nc.vector.tensor_scalar_max(sc[:, :], sc[:, :], -10.0)
            ex = pp.tile([P, t * SLOTS], F32)
            nc.scalar.activation(ex[:, :], sc[:, :], mybir.ActivationFunctionType.Exp)
            lnout = pp.tile([P, t * SLOTS], F32)
            loss = pp.tile([P, 1], F32)
            nc.scalar.activation(
                lnout[:, :],
                ex[:, :],
                mybir.ActivationFunctionType.Ln,
                bias=1.0,
                accum_out=loss[:, :],
            )
            nc.sync.dma_start(out.ap()[:, :], loss[:, :])

    nc.compile()
    return nc


def _prep_core_inputs(emb0, emb1, word_idx, ctx_inds, ctx_lens, neg_inds, m, t):
    bc = P * t
    sl = slice(m * bc, (m + 1) * bc)
    ctx = np.ascontiguousarray(
        ctx_inds[sl].astype(np.int32).reshape(P, t * C)
    )
    wn = np.concatenate(
        [
            word_idx[sl].astype(np.int32).reshape(P, t, 1),
            neg_inds[sl].astype(np.int32).reshape(P, t, NNEG),
        ],
        axis=2,
    ).reshape(P, t * SLOTS)
    ln = np.ascontiguousarray(ctx_lens[sl].astype(np.float32).reshape(P, t))
    return {
        "emb0": emb0,
        "emb1": emb1,
        "ctx_idx": ctx,
        "wn_idx": np.ascontiguousarray(wn),
        "lens": ln,
    }


_NC_CACHE = {}


def _get_nc():
    if "nc" not in _NC_CACHE:
        _NC_CACHE["nc"] = build_nc()
    return _NC_CACHE["nc"]


def kernel(emb0, emb1, word_idx, ctx_inds, ctx_lens, neg_inds):
    emb0 = np.ascontiguousarray(np.asarray(emb0, dtype=np.float32))
    emb1 = np.ascontiguousarray(np.asarray(emb1, dtype=np.float32))
    word_idx = np.asarray(word_idx)
    ctx_inds = np.asarray(ctx_inds)
    ctx_lens = np.asarray(ctx_lens)
    neg_inds = np.asarray(neg_inds)

    nc = _get_nc()
    in_maps = [
        _prep_core_inputs(emb0, emb1, word_idx, ctx_inds, ctx_lens, neg_inds, m, T)
        for m in range(M)
    ]
    res = run_bass_kernel_spmd(nc, in_maps, core_ids=list(range(M)))
    total = np.float64(0.0)
    for r in res.results:
        total += np.float64(r["out"].sum(dtype=np.float64))
    return np.array(total, dtype=np.float32)
